# revision 4
# baseline (speedup 1.0000x reference)
"""CoulombQeq Trainium2 kernel.

Computes, for each of B=64 graphs:
  chi = MLP(node_feats)            (tanhshrink MLP, dims 128-64-64-1)
  A   = coulomb matrix             (erf-screened 1/r, SPD, 512x512)
  solve bordered system [[A, 1], [1^T, 0]] [q; lam] = [-chi; Q]
Returns (charges [64,512], A [64,512,512]).

Strategy: data-parallel over 8 NeuronCores (8 graphs each). On device:
  - distances via Gram matrix (PE matmul, K=3)
  - A assembly with ACT erf; diagonal overwritten analytically (affine_select)
  - solve: block Gaussian elimination on the 4x4 grid of 128-blocks;
    each 128x128 SPD pivot inverted by Newton-Schulz iteration
    (X <- X(2I - S X)) seeded with I/gershgorin(S); all heavy lifting
    is PE matmuls. Bordered system reduced via two RHS (y1 = A^-1(-chi),
    y2 = A^-1 1): q = y1 - lam*y2, lam = (1^T y1 - Q)/(1^T y2).
"""

import numpy as np

import concourse.bacc as bacc
import concourse.mybir as mybir
import concourse.tile as tile
from concourse import bass_isa, bass_utils
from concourse.bass import ts
from concourse.masks import make_identity

F32 = mybir.dt.float32
AF = mybir.ActivationFunctionType
OP = mybir.AluOpType

B, N, D, U = 64, 512, 128, 64
NCORES = 8
GPC = B // NCORES  # graphs per core
RB = N // 128  # row blocks per graph
KCOUL = 14.399645351950548

# Newton-Schulz iteration counts per pivot block (block 0 has kappa~55-70,
# later Schur-complement pivots kappa~3)
NS_ITERS = (12, 8, 8, 8)

_CACHE: dict = {}


def _build(sigma0: float, hard0: float, b1z: bool, b2z: bool, b3val: float):
    """Emit + compile the SPMD program (same on all 8 cores)."""
    nc = bacc.Bacc("TRN2", target_bir_lowering=False, debug=False,
                   num_devices=NCORES)

    nfT_d = nc.dram_tensor("nfT", [GPC, D, N], F32, kind="ExternalInput")
    posT_d = nc.dram_tensor("posT", [GPC, 3, N], F32, kind="ExternalInput")
    W1_d = nc.dram_tensor("W1", [D, U], F32, kind="ExternalInput")
    W2_d = nc.dram_tensor("W2", [U, U], F32, kind="ExternalInput")
    W3_d = nc.dram_tensor("W3", [U, 1], F32, kind="ExternalInput")
    b1_d = nc.dram_tensor("b1", [U, 1], F32, kind="ExternalInput")
    b2_d = nc.dram_tensor("b2", [U, 1], F32, kind="ExternalInput")
    tc_d = nc.dram_tensor("tc8", [1, GPC], F32, kind="ExternalInput")
    A_d = nc.dram_tensor("Aout", [GPC, N, N], F32, kind="ExternalOutput")
    q_d = nc.dram_tensor("qout", [GPC, N], F32, kind="ExternalOutput")

    # baked constants (depend only on sigma0/hard0 scalars)
    gii = np.float64(np.sqrt(2.0) * sigma0)
    diagconst = float(
        KCOUL * (2.0 / np.sqrt(np.pi)) / (np.sqrt(2.0) * gii)
        + hard0
        + KCOUL / (np.sqrt(np.pi) * gii)
    )
    sconst = float(1.0 / (4.0 * sigma0 * sigma0))  # u^2 = d2 / (2 gamma^2)
    c_a = float(KCOUL / (2.0 * sigma0))  # a = c_a * erf(u) / u

    with tile.TileContext(nc) as tc:
        with (
            tc.tile_pool(name="consts", bufs=1) as cpool,
            tc.tile_pool(name="sb", bufs=2) as sb,
            tc.tile_pool(name="sbA", bufs=2) as sbA,
            tc.tile_pool(name="sbW", bufs=2) as sbW,
            tc.tile_pool(name="sbX", bufs=3) as sbX,
            tc.tile_pool(name="pbig", bufs=3, space="PSUM") as pbig,
            tc.tile_pool(name="pns", bufs=2, space="PSUM") as pns,
            tc.tile_pool(name="psm", bufs=3, space="PSUM") as psm,
        ):
            ident = cpool.tile([128, 128], F32)
            make_identity(nc, ident)
            twoI = cpool.tile([128, 128], F32)
            nc.vector.tensor_scalar_mul(twoI[:], ident[:], 2.0)
            ones3 = cpool.tile([3, 1], F32)
            nc.vector.memset(ones3[:], 1.0)
            ones128 = cpool.tile([128, 1], F32)
            nc.vector.memset(ones128[:], 1.0)
            ones_r = cpool.tile([1, 128], F32)
            nc.vector.memset(ones_r[:], 1.0)
            w1sb = cpool.tile([D, U], F32)
            nc.sync.dma_start(w1sb[:], W1_d.ap())
            w2sb = cpool.tile([U, U], F32)
            nc.sync.dma_start(w2sb[:], W2_d.ap())
            w3sb = cpool.tile([U, 1], F32)
            nc.sync.dma_start(w3sb[:], W3_d.ap())
            tcsb = cpool.tile([1, GPC], F32)
            nc.sync.dma_start(tcsb[:], tc_d.ap())
            b1sb = cpool.tile([U, 1], F32)
            b2sb = cpool.tile([U, 1], F32)
            if not b1z:
                nc.sync.dma_start(b1sb[:], b1_d.ap())
            if not b2z:
                nc.sync.dma_start(b2sb[:], b2_d.ap())

            for g in range(GPC):
                # ---------------- MLP: rhs col0 = -chi ----------------
                xT = sb.tile([D, N], F32, tag="xT")
                nc.sync.dma_start(xT[:], nfT_d.ap()[g])
                ps1 = pbig.tile([U, N], F32, tag="pbig")
                nc.tensor.matmul(ps1[:], w1sb[:], xT[:], start=True, stop=True)
                h1 = sb.tile([U, N], F32, tag="h1")
                if b1z:
                    t1 = sb.tile([U, N], F32, tag="t1")
                    nc.scalar.activation(t1[:], ps1[:], AF.Tanh)
                    nc.vector.tensor_sub(h1[:], ps1[:], t1[:])
                else:
                    xb = sb.tile([U, N], F32, tag="t1")
                    nc.vector.tensor_scalar_add(xb[:], ps1[:], b1sb[:, 0:1])
                    t1 = sb.tile([U, N], F32, tag="t1b")
                    nc.scalar.activation(t1[:], xb[:], AF.Tanh)
                    nc.vector.tensor_sub(h1[:], xb[:], t1[:])
                ps2 = pbig.tile([U, N], F32, tag="pbig")
                nc.tensor.matmul(ps2[:], w2sb[:], h1[:], start=True, stop=True)
                h2 = sb.tile([U, N], F32, tag="h2")
                if b2z:
                    t2 = sb.tile([U, N], F32, tag="t1")
                    nc.scalar.activation(t2[:], ps2[:], AF.Tanh)
                    nc.vector.tensor_sub(h2[:], ps2[:], t2[:])
                else:
                    xb = sb.tile([U, N], F32, tag="t1")
                    nc.vector.tensor_scalar_add(xb[:], ps2[:], b2sb[:, 0:1])
                    t2 = sb.tile([U, N], F32, tag="t1b")
                    nc.scalar.activation(t2[:], xb[:], AF.Tanh)
                    nc.vector.tensor_sub(h2[:], xb[:], t2[:])

                rhs = [sbW.tile([128, 2], F32, name=f"rhs{rb}_{g}", tag=f"rhs{rb}") for rb in range(RB)]
                for rb in range(RB):
                    psc = psm.tile([128, 1], F32, tag="psm")
                    nc.tensor.matmul(psc[:], h2[:, ts(rb, 128)], w3sb[:],
                                     start=True, stop=True)
                    # y (pre-activation chi) = psc + b3; chi = y - tanh(y)
                    # rhs0 = -chi = tanh(y) - y
                    tch = sbW.tile([128, 1], F32, tag="tch")
                    if b3val == 0.0:
                        nc.scalar.activation(tch[:], psc[:], AF.Tanh)
                        nc.vector.tensor_sub(rhs[rb][:, 0:1], tch[:], psc[:])
                    else:
                        yb = sbW.tile([128, 1], F32, tag="yb")
                        nc.vector.tensor_scalar_add(yb[:], psc[:], float(b3val))
                        nc.scalar.activation(tch[:], yb[:], AF.Tanh)
                        nc.vector.tensor_sub(rhs[rb][:, 0:1], tch[:], yb[:])
                    nc.vector.memset(rhs[rb][:, 1:2], 1.0)

                # ---------------- A assembly ----------------
                pT = sb.tile([3, N], F32, tag="pT")
                nc.sync.dma_start(pT[:], posT_d.ap()[g])
                sq = sb.tile([3, N], F32, tag="sq")
                nc.vector.tensor_mul(sq[:], pT[:], pT[:])
                nps = psm.tile([1, N], F32, tag="psm")
                nc.tensor.matmul(nps[:], ones3[:], sq[:], start=True, stop=True)
                nrow = sb.tile([1, N], F32, tag="nrow")
                nc.scalar.copy(nrow[:], nps[:])
                nbps = pbig.tile([128, N], F32, tag="pbig")
                nc.tensor.matmul(nbps[:], ones_r[:], nrow[:], start=True, stop=True)
                NB = sb.tile([128, N], F32, tag="NB")
                nc.scalar.copy(NB[:], nbps[:])
                ncols = sbW.tile([128, RB], F32, tag="ncols")
                for rb in range(RB):
                    ncp = psm.tile([128, 1], F32, tag="psm")
                    nc.tensor.matmul(ncp[:], sq[:, ts(rb, 128)], ones3[:],
                                     start=True, stop=True)
                    nc.vector.tensor_copy(ncols[:, rb : rb + 1], ncp[:])

                Ablk = [sbA.tile([128, N], F32, name=f"A{rb}_{g}", tag=f"A{rb}") for rb in range(RB)]
                for rb in range(RB):
                    gps = pbig.tile([128, N], F32, tag="pbig")
                    nc.tensor.matmul(gps[:], pT[:, ts(rb, 128)], pT[:],
                                     start=True, stop=True)
                    d2 = sb.tile([128, N], F32, tag="d2")
                    nc.vector.scalar_tensor_tensor(
                        d2[:], gps[:], -2.0, NB[:], op0=OP.mult, op1=OP.add)
                    nc.vector.tensor_scalar_add(d2[:], d2[:], ncols[:, rb : rb + 1])
                    u = sb.tile([128, N], F32, tag="u")
                    nc.scalar.activation(u[:], d2[:], AF.Sqrt, scale=sconst)
                    e = sb.tile([128, N], F32, tag="e")
                    nc.scalar.activation(e[:], u[:], AF.Erf)
                    ru = sb.tile([128, N], F32, tag="ru")
                    nc.vector.reciprocal(ru[:], u[:])
                    nc.vector.scalar_tensor_tensor(
                        Ablk[rb][:], e[:], c_a, ru[:], op0=OP.mult, op1=OP.mult)
                    nc.gpsimd.affine_select(
                        Ablk[rb][:], Ablk[rb][:],
                        pattern=[[1, N]], base=-(rb * 128), channel_multiplier=-1,
                        compare_op=OP.not_equal, fill=diagconst)
                    nc.sync.dma_start(A_d.ap()[g, ts(rb, 128)], Ablk[rb][:])

                # ---------------- block elimination ----------------
                Wblk = [sbW.tile([128, 128], F32, name=f"W{kk}_{g}", tag=f"W{kk}") for kk in range(RB)]
                for kk in range(RB):
                    S = Ablk[kk][:, ts(kk, 128)]
                    rs = sbW.tile([128, 1], F32, tag="rs")
                    nc.vector.tensor_reduce(
                        rs[:], S, mybir.AxisListType.X, OP.add,
                        apply_absolute_value=True)
                    lam = sbW.tile([128, 1], F32, tag="lam")
                    nc.gpsimd.partition_all_reduce(
                        lam[:], rs[:], 128, bass_isa.ReduceOp.max)
                    rcol = sbW.tile([128, 1], F32, tag="rcol")
                    nc.vector.reciprocal(rcol[:], lam[:])
                    X = sbX.tile([128, 128], F32, tag="nsX")
                    nc.vector.tensor_scalar_mul(X[:], ident[:], rcol[:])
                    for it in range(NS_ITERS[kk]):
                        yps = pns.tile([128, 128], F32, tag="pns")
                        nc.tensor.matmul(yps[:], S, X[:], start=True, stop=True)
                        Z = sbX.tile([128, 128], F32, tag="nsZ")
                        nc.vector.scalar_tensor_tensor(
                            Z[:], yps[:], -1.0, twoI[:], op0=OP.mult, op1=OP.add)
                        xps = pns.tile([128, 128], F32, tag="pns")
                        nc.tensor.matmul(xps[:], X[:], Z[:], start=True, stop=True)
                        last = it == NS_ITERS[kk] - 1
                        Xn = Wblk[kk] if last else sbX.tile([128, 128], F32, tag="nsX")
                        nc.scalar.copy(Xn[:], xps[:])
                        X = Xn
                    if kk < RB - 1:
                        wid = (RB - 1 - kk) * 128
                        for i in range(kk + 1, RB):
                            mps = pns.tile([128, 128], F32, tag="pns")
                            nc.tensor.matmul(mps[:], Wblk[kk][:],
                                             Ablk[kk][:, ts(i, 128)],
                                             start=True, stop=True)
                            Mi = sbX.tile([128, 128], F32, tag="Mi")
                            nc.scalar.copy(Mi[:], mps[:])
                            u1 = pbig.tile([128, wid], F32, tag="pbig")
                            nc.tensor.matmul(
                                u1[:], Mi[:],
                                Ablk[kk][:, (kk + 1) * 128 : N],
                                start=True, stop=True)
                            nc.vector.tensor_sub(
                                Ablk[i][:, (kk + 1) * 128 : N],
                                Ablk[i][:, (kk + 1) * 128 : N], u1[:])
                            u2 = psm.tile([128, 2], F32, tag="psm")
                            nc.tensor.matmul(u2[:], Mi[:], rhs[kk][:],
                                             start=True, stop=True)
                            nc.vector.tensor_sub(rhs[i][:], rhs[i][:], u2[:])

                # ---------------- back substitution ----------------
                xsol = [sbW.tile([128, 2], F32, name=f"x{kk}_{g}", tag=f"x{kk}") for kk in range(RB)]
                for kk in range(RB - 1, -1, -1):
                    if kk < RB - 1:
                        acc = psm.tile([128, 2], F32, tag="psm")
                        for j in range(kk + 1, RB):
                            nc.tensor.matmul(
                                acc[:], Ablk[j][:, ts(kk, 128)], xsol[j][:],
                                start=(j == kk + 1), stop=(j == RB - 1))
                        s = sbW.tile([128, 2], F32, tag="s")
                        nc.vector.tensor_sub(s[:], rhs[kk][:], acc[:])
                    else:
                        s = rhs[kk]
                    xps2 = psm.tile([128, 2], F32, tag="psm")
                    nc.tensor.matmul(xps2[:], Wblk[kk][:], s[:],
                                     start=True, stop=True)
                    nc.scalar.copy(xsol[kk][:], xps2[:])

                sums = psm.tile([1, 2], F32, tag="psm")
                for kk in range(RB):
                    nc.tensor.matmul(sums[:], ones128[:], xsol[kk][:],
                                     start=(kk == 0), stop=(kk == RB - 1))
                ssb = sbW.tile([1, 2], F32, tag="ssb")
                nc.vector.tensor_copy(ssb[:], sums[:])
                num = sbW.tile([1, 1], F32, tag="num")
                nc.vector.tensor_sub(num[:], tcsb[:, g : g + 1], ssb[:, 0:1])
                r2 = sbW.tile([1, 1], F32, tag="r2")
                nc.vector.reciprocal(r2[:], ssb[:, 1:2])
                nlam = sbW.tile([1, 1], F32, tag="nlam")
                nc.vector.tensor_mul(nlam[:], num[:], r2[:])
                nlps = psm.tile([128, 1], F32, tag="psm")
                nc.tensor.matmul(nlps[:], ones_r[:], nlam[:], start=True, stop=True)
                nlcol = sbW.tile([128, 1], F32, tag="nlcol")
                nc.vector.tensor_copy(nlcol[:], nlps[:])
                for kk in range(RB):
                    qrb = sbW.tile([128, 1], F32, tag="qrb")
                    nc.vector.scalar_tensor_tensor(
                        qrb[:], xsol[kk][:, 1:2], nlcol[:], xsol[kk][:, 0:1],
                        op0=OP.mult, op1=OP.add)
                    nc.sync.dma_start(q_d.ap()[g, ts(kk, 128)], qrb[:])

    nc.compile()
    return nc


def _get_program(sigma0, hard0, b1z, b2z, b3val):
    key = (round(float(sigma0), 12), round(float(hard0), 12), b1z, b2z,
           round(float(b3val), 12))
    if key not in _CACHE:
        _CACHE[key] = _build(*key)
    return _CACHE[key]


def _numpy_fallback(pos, node_feats, node_type, total_charge, hardness, sigma,
                    W1, b1, W2, b2, W3, b3):
    """CPU reference path for inputs outside the kernel's fast path
    (non-uniform sigma/hardness). Mirrors the jax reference in numpy."""
    from scipy.special import erf as _erf

    f32 = np.float32
    k = f32(KCOUL)
    h = node_feats @ W1 + b1
    h = h - np.tanh(h)
    h = h @ W2 + b2
    h = h - np.tanh(h)
    h = h @ W3 + b3
    chi = h[..., 0].astype(f32)
    hard = hardness[node_type].astype(f32)
    sig = sigma[node_type].astype(f32)
    diff = pos[:, :, None, :] - pos[:, None, :, :]
    bd = np.sqrt((diff * diff).sum(-1)).astype(f32) + f32(1e-8)
    gam = np.sqrt(sig[:, :, None] ** 2 + sig[:, None, :] ** 2).astype(f32)
    a = (k * _erf(bd / (np.sqrt(f32(2.0)) * gam)) / bd).astype(f32)
    gii = (np.sqrt(f32(2.0)) * sig).astype(f32)
    diag = (hard + k / (np.sqrt(f32(np.pi)) * gii)).astype(f32)
    A = a.copy()
    idx = np.arange(N)
    A[:, idx, idx] += diag
    Bm = np.ones((B, N + 1, N + 1), dtype=f32)
    Bm[:, :N, :N] = A
    Bm[:, N, N] = 0.0
    rhs = np.concatenate([-chi, total_charge[:, None].astype(f32)], axis=1)
    x = np.linalg.solve(Bm.astype(np.float64), rhs.astype(np.float64)[..., None])[..., 0]
    return x[:, :N].astype(f32), A


def kernel(**inputs):
    pos = np.ascontiguousarray(np.asarray(inputs["pos"], dtype=np.float32))
    node_feats = np.asarray(inputs["node_feats"], dtype=np.float32)
    node_type = np.asarray(inputs["node_type"])
    total_charge = np.asarray(inputs["total_charge"], dtype=np.float32)
    hardness = np.asarray(inputs["hardness"], dtype=np.float32)
    sigma = np.asarray(inputs["sigma"], dtype=np.float32)
    W1 = np.ascontiguousarray(np.asarray(inputs["W1"], dtype=np.float32))
    b1 = np.asarray(inputs["b1"], dtype=np.float32)
    W2 = np.ascontiguousarray(np.asarray(inputs["W2"], dtype=np.float32))
    b2 = np.asarray(inputs["b2"], dtype=np.float32)
    W3 = np.ascontiguousarray(np.asarray(inputs["W3"], dtype=np.float32))
    b3 = np.asarray(inputs["b3"], dtype=np.float32)

    sig_nodes = sigma[node_type]
    hard_nodes = hardness[node_type]
    uniform = (
        np.all(sig_nodes == sig_nodes.flat[0])
        and np.all(hard_nodes == hard_nodes.flat[0])
        and sig_nodes.flat[0] > 0
    )
    if not uniform:
        return _numpy_fallback(pos, node_feats, node_type, total_charge,
                               hardness, sigma, W1, b1, W2, b2, W3, b3)

    sigma0 = float(sig_nodes.flat[0])
    hard0 = float(hard_nodes.flat[0])
    b1z = not np.any(b1)
    b2z = not np.any(b2)
    b3val = float(b3.reshape(-1)[0])

    nc = _get_program(sigma0, hard0, b1z, b2z, b3val)

    nfT = np.ascontiguousarray(np.transpose(node_feats, (0, 2, 1)))  # [B,D,N]
    posT = np.ascontiguousarray(np.transpose(pos, (0, 2, 1)))  # [B,3,N]
    W3c = W3.reshape(U, 1)
    b1c = b1.reshape(U, 1)
    b2c = b2.reshape(U, 1)

    in_maps = []
    for c in range(NCORES):
        sl = slice(c * GPC, (c + 1) * GPC)
        in_maps.append({
            "nfT": nfT[sl],
            "posT": posT[sl],
            "W1": W1,
            "W2": W2,
            "W3": W3c,
            "b1": b1c,
            "b2": b2c,
            "tc8": total_charge[sl].reshape(1, GPC),
        })

    res = bass_utils.run_bass_kernel_spmd(nc, in_maps,
                                          core_ids=list(range(NCORES)))
    charges = np.empty((B, N), dtype=np.float32)
    A = np.empty((B, N, N), dtype=np.float32)
    for c in range(NCORES):
        sl = slice(c * GPC, (c + 1) * GPC)
        charges[sl] = res.results[c]["qout"]
        A[sl] = res.results[c]["Aout"]
    return charges, A
